# revision 1
# baseline (speedup 1.0000x reference)
"""ConvolvedAttention (sliding-window causal attention, W=33) on 8 TRN2 NeuronCores.

Sharding: sequence L=8192 split 8 ways (1024 tokens/core), data-parallel over
cores. Host passes each core its query shard plus key/value shards with a
32-token halo on the left; projections are replicated. Each core runs a fused
Bass/Tile kernel: qkv projections -> banded scores (k-major, query-aligned
128-key supers) -> masked softmax -> AV -> out-projection. Host folds in the
output biases and reassembles.
"""

import numpy as np

# ---- problem constants (hardcoded per contract) ----
L, N, E = 8192, 2, 256
H, HD = 8, 32
WHALF = 32            # window//2 ; attended span = 33 (past only)
NCORES = 8
T = L // NCORES       # 1024 tokens per core
TL = 128 + T          # local tokens per batch entry: 96 pad + 32 halo + 1024
NEG = -1e9
NSUP = 9              # supers 0..8 ; super 0 = pad+halo block

# wpack column layout (fp32 cols per partition)
_WQ = 0               # 4 tiles [128,128]  (ki*2+ko)
_WK = 512
_WV = 1024            # 2 tiles [128,256]  (ki)
_WO = 1536            # 2 tiles [128,256]  (g = E_in chunk)
_BQ = 2048            # 2 cols  (ko)
_BK = 2050            # 2 cols
_BD = 2052            # [8,256] block-diag indicator (2 groups of 128 cols)
_I128 = 2308          # [128,128] identity
_MMAIN = 2436         # [128,320] band mask, 2 heads tiled (additive 0/-1e9)
_M0 = 2756            # [128,64] super-0 mask (pad+halo), 2 heads tiled
_ONES32 = 2820        # [128,32] all-ones (S-sum lhsT)
_WPCOLS = 2852

_STATE = {}


def _sup_w(s):
    return 32 if s == 0 else (128 if s == NSUP - 1 else 160)


def _build_program():
    import os
    level = int(os.environ.get("KBUILD_LEVEL", "9"))
    import concourse.bacc as bacc
    import concourse.tile as tile
    import concourse.mybir as mybir
    from contextlib import ExitStack

    f32 = mybir.dt.float32
    AF = mybir.ActivationFunctionType

    nc = bacc.Bacc("TRN2", target_bir_lowering=False, debug=False)
    xq_d = nc.declare_dram_parameter("xq", [2, 128, 2 * T], f32, isOutput=False)
    xk_d = nc.declare_dram_parameter("xk", [2, 128, 2 * TL], f32, isOutput=False)
    xv_d = nc.declare_dram_parameter("xv", [2, 128, 2 * TL], f32, isOutput=False)
    wp_d = nc.declare_dram_parameter("wpack", [128, _WPCOLS], f32, isOutput=False)
    out_d = nc.declare_dram_parameter("out", [2, 8, 128, 256], f32, isOutput=True)

    ones_col = nc.const_aps.tensor(1.0, (128, 1))

    with ExitStack() as stk:
        tc = stk.enter_context(tile.TileContext(nc))
        sb = stk.enter_context(tc.tile_pool(name="sb", bufs=1))
        sb_probs = stk.enter_context(tc.tile_pool(name="probs", bufs=2))
        sb_tr = stk.enter_context(tc.tile_pool(name="tr", bufs=3))

        # ---- load inputs ----
        wp = sb.tile([128, _WPCOLS], f32, tag="wp")
        nc.sync.dma_start(wp[:], wp_d[:])
        xq = []
        xk = []
        xv = []
        for ki in range(2):
            t_q = sb.tile([128, 2 * T], f32, tag=f"xq{ki}", name=f"xq{ki}")
            nc.sync.dma_start(t_q[:], xq_d[ki])
            xq.append(t_q)
            t_k = sb.tile([128, 2 * TL], f32, tag=f"xk{ki}", name=f"xk{ki}")
            nc.sync.dma_start(t_k[:], xk_d[ki])
            xk.append(t_k)
            t_v = sb.tile([128, 2 * TL], f32, tag=f"xv{ki}", name=f"xv{ki}")
            nc.sync.dma_start(t_v[:], xv_d[ki])
            xv.append(t_v)

        q_sb = [sb.tile([128, 2 * T], f32, tag=f"q{ko}", name=f"q{ko}") for ko in range(2)]
        k_sb = [sb.tile([128, 2 * TL], f32, tag=f"k{ko}", name=f"k{ko}") for ko in range(2)]
        v_sb = [sb.tile([128, 256], f32, tag=f"v{b}", name=f"v{b}") for b in range(2 * NSUP)]

        # ---- phase 1: projections ----
        with tc.tile_pool(name="pp", bufs=3, space="PSUM") as pp:
            # q / k projections: out [E_out chunk, tokens]
            for ko in range(2):
                bq_ap = wp[:, _BQ + ko : _BQ + ko + 1]
                bk_ap = wp[:, _BK + ko : _BK + ko + 1]
                for g0 in range(0, 2 * T, 512):
                    ps = pp.tile([128, 512], f32, tag="pq", name="pq")
                    for ki in range(2):
                        nc.tensor.matmul(
                            ps[:],
                            wp[:, _WQ + (ki * 2 + ko) * 128 : _WQ + (ki * 2 + ko + 1) * 128],
                            xq[ki][:, g0 : g0 + 512],
                            start=(ki == 0),
                            stop=(ki == 1),
                        )
                    nc.scalar.activation(
                        q_sb[ko][:, g0 : g0 + 512], ps[:], AF.Identity, bias=bq_ap
                    )
                for g0 in range(0, 2 * TL, 512):
                    w = min(512, 2 * TL - g0)
                    ps = pp.tile([128, 512], f32, tag="pq", name="pq")
                    for ki in range(2):
                        nc.tensor.matmul(
                            ps[:, :w],
                            wp[:, _WK + (ki * 2 + ko) * 128 : _WK + (ki * 2 + ko + 1) * 128],
                            xk[ki][:, g0 : g0 + w],
                            start=(ki == 0),
                            stop=(ki == 1),
                        )
                    nc.scalar.activation(
                        k_sb[ko][:, g0 : g0 + w], ps[:, :w], AF.Identity, bias=bk_ap
                    )
            # v projection: out [tokens, E_out]
            for b in range(2 * NSUP):
                ps = pp.tile([128, 256], f32, tag="pv", name="pv")
                for ki in range(2):
                    nc.tensor.matmul(
                        ps[:],
                        xv[ki][:, b * 128 : (b + 1) * 128],
                        wp[:, _WV + ki * 256 : _WV + (ki + 1) * 256],
                        start=(ki == 0),
                        stop=(ki == 1),
                    )
                nc.vector.tensor_copy(v_sb[b][:], ps[:])

        # ---- phase 2: attention ----
        i128 = wp[:, _I128 : _I128 + 128]
        mmain = wp[:, _MMAIN : _MMAIN + 320].rearrange("p (t w) -> p t w", t=2)
        m0 = wp[:, _M0 : _M0 + 64]

        with (
            tc.tile_pool(name="psc", bufs=4, space="PSUM") as psc,
            tc.tile_pool(name="pav", bufs=2, space="PSUM") as pav,
            tc.tile_pool(name="pms", bufs=2, space="PSUM") as pms,
        ):
            for n in range(2):
                probs = {}
                for s in range(NSUP):
                    w = _sup_w(s)
                    qs = 0 if s == 0 else 128 * (s - 1)
                    pr = sb_probs.tile([128, 8 * 160], f32, tag="probs", name="probs")
                    probs[s] = pr
                    if level < 2:
                        continue
                    pr_r = pr[:, : 8 * w].rearrange("p (a b) -> p a b", a=8)
                    for j in range(4):
                        # bank j: heads j and j+4, both at row group 32j
                        sc = psc.tile([128, 2 * 160], f32, tag="sc", name="sc")
                        if s == 0:
                            nc.tensor.matmul(
                                sc[:, : 2 * w], i128, m0,
                                start=True, stop=False, skip_group_check=True,
                            )
                        else:
                            nc.tensor.matmul(
                                sc[:, : 2 * w], i128, mmain[:, :, :w],
                                start=True, stop=False, skip_group_check=True,
                            )
                        for hb in range(2):
                            h = j + 4 * hb
                            ch, hr = hb, 32 * j
                            nc.tensor.matmul(
                                sc[:, hb * w : (hb + 1) * w],
                                k_sb[ch][hr : hr + 32, n * TL + 128 * s : n * TL + 128 * s + 128],
                                q_sb[ch][hr : hr + 32, n * T + qs : n * T + qs + w],
                                start=False, stop=(hb == 1),
                                tile_position=(hr, 0), skip_group_check=True,
                            )
                        nc.scalar.activation(
                            pr_r[:, j::4, :], sc[:, : 2 * w], AF.Exp,
                        )
                    if s == 0 or level < 3:
                        continue
                    # finalize query block a = s-1 (queries 128a .. 128a+128)
                    a = s - 1
                    wp_prev = _sup_w(s - 1)
                    wc = min(w, 128)
                    pcur, pprev = probs[s], probs[s - 1]
                    sps = pms.tile([128, 256], f32, tag="ms", name="ms")
                    ones32 = wp[:, _ONES32 : _ONES32 + 32]
                    for h in range(8):
                        hp, hc = 32 * (h % 4), 128 * (h // 4)
                        nc.tensor.matmul(
                            sps[hp : hp + 32, hc : hc + wc], ones32,
                            pcur[:, h * w : h * w + wc],
                            start=True, stop=False, skip_group_check=True,
                            tile_position=(0, hp),
                        )
                        nc.tensor.matmul(
                            sps[hp : hp + 32, hc : hc + 32], ones32,
                            pprev[:, h * wp_prev + wp_prev - 32 : h * wp_prev + wp_prev],
                            start=False, stop=True, skip_group_check=True,
                            tile_position=(0, hp),
                        )
                    s_sb = sb_tr.tile([128, 256], f32, tag="ssb", name="ssb")
                    nc.vector.tensor_copy(s_sb[:], sps[:])
                    s_r = sb_tr.tile([128, 256], f32, tag="sr", name="sr")
                    nc.vector.reciprocal_approx_fast(out=s_r[:], in_=s_sb[:])
                    if level < 4:
                        o_sb = sb_tr.tile([128, 256], f32, tag="osb", name="osb")
                        nc.vector.tensor_copy(o_sb[:], s_r[:])
                        nc.sync.dma_start(out_d[n, a], o_sb[:])
                        continue
                    avn = []
                    for g in range(2):
                        av = pav.tile([128, 128], f32, tag="av", name="av")
                        for hb in range(4):
                            h = 4 * g + hb
                            hr = 32 * hb
                            nc.tensor.matmul(
                                av[hr : hr + 32, :wc],
                                v_sb[NSUP * n + s][:, 32 * h : 32 * h + 32],
                                pcur[:, h * w : h * w + wc],
                                start=True, stop=False,
                                tile_position=(0, hr), skip_group_check=True,
                            )
                            nc.tensor.matmul(
                                av[hr : hr + 32, :32],
                                v_sb[NSUP * n + s - 1][:, 32 * h : 32 * h + 32],
                                pprev[:, h * wp_prev + wp_prev - 32 : h * wp_prev + wp_prev],
                                start=False, stop=True,
                                tile_position=(0, hr), skip_group_check=True,
                            )
                        t_avn = sb_tr.tile([128, 128], f32, tag="avn", name="avn")
                        if level >= 5:
                            nc.vector.tensor_mul(t_avn[:], av[:], s_r[:, 128 * g : 128 * (g + 1)])
                        else:
                            nc.vector.tensor_copy(t_avn[:], av[:])
                        avn.append(t_avn)
                    op = pms.tile([128, 256], f32, tag="ms", name="ms")
                    for g in range(2):
                        nc.tensor.matmul(
                            op[:], avn[g][:],
                            wp[:, _WO + g * 256 : _WO + (g + 1) * 256],
                            start=(g == 0), stop=(g == 1),
                        )
                    o_sb = sb_tr.tile([128, 256], f32, tag="osb", name="osb")
                    nc.scalar.copy(o_sb[:], op[:])
                    nc.sync.dma_start(out_d[n, a], o_sb[:])
                    del probs[s - 1]
                if level < 3:
                    for a in range(8):
                        o_sb = sb_tr.tile([128, 256], f32, tag="osb", name="osb")
                        if level >= 2:
                            nc.vector.tensor_copy(o_sb[:], probs[a][:, :256])
                        else:
                            nc.vector.tensor_copy(o_sb[:], v_sb[a][:])
                        nc.sync.dma_start(out_d[n, a], o_sb[:])
    nc.compile()
    return nc


def _host_prep(query, key, value, in_proj_w, in_proj_b, out_proj_w, out_proj_b):
    """Build per-core input maps + the host-side output bias vector."""
    s = 1.0 / np.sqrt(HD)
    wq = (in_proj_w[:E] * s).astype(np.float32)
    wk = in_proj_w[E : 2 * E].astype(np.float32)
    wv = in_proj_w[2 * E :].astype(np.float32)
    bq = (in_proj_b[:E] * s).astype(np.float32)
    bk = in_proj_b[E : 2 * E].astype(np.float32)
    bv = in_proj_b[2 * E :].astype(np.float32)
    wo = out_proj_w.astype(np.float32)

    wpack_base = np.zeros((128, _WPCOLS), np.float32)
    wqT, wkT = wq.T.copy(), wk.T.copy()   # [E_in, E_out]
    for ki in range(2):
        for ko in range(2):
            wpack_base[:, _WQ + (ki * 2 + ko) * 128 : _WQ + (ki * 2 + ko + 1) * 128] = \
                wqT[ki * 128 : (ki + 1) * 128, ko * 128 : (ko + 1) * 128]
            wpack_base[:, _WK + (ki * 2 + ko) * 128 : _WK + (ki * 2 + ko + 1) * 128] = \
                wkT[ki * 128 : (ki + 1) * 128, ko * 128 : (ko + 1) * 128]
        wpack_base[:, _WV + ki * 256 : _WV + (ki + 1) * 256] = \
            wv.T[ki * 128 : (ki + 1) * 128, :]
        wpack_base[:, _WO + ki * 256 : _WO + (ki + 1) * 256] = \
            wo.T[ki * 128 : (ki + 1) * 128, :]
    for ko in range(2):
        wpack_base[:, _BQ + ko] = bq[ko * 128 : (ko + 1) * 128]
        wpack_base[:, _BK + ko] = bk[ko * 128 : (ko + 1) * 128]
    # block-diag indicator [8, 256]: row k, col 128g+p -> 1 iff k == 4g + p//32
    for g in range(2):
        for hh in range(4):
            wpack_base[4 * g + hh, _BD + 128 * g + 32 * hh : _BD + 128 * g + 32 * (hh + 1)] = 1.0
    wpack_base[:128, _I128 : _I128 + 128] = np.eye(128, dtype=np.float32)
    wpack_base[:, _ONES32 : _ONES32 + 32] = 1.0
    # band mask [128, 2x160]: valid iff 0 <= c - rho <= WHALF
    rho = np.arange(128)[:, None]
    c = np.arange(160)[None, :]
    band = np.where((c - rho >= 0) & (c - rho <= WHALF), 0.0, NEG).astype(np.float32)
    wpack_base[:, _MMAIN : _MMAIN + 160] = band
    wpack_base[:, _MMAIN + 160 : _MMAIN + 320] = band

    # super-0 mask [128, 2x32]: rows 0..96 pad -> NEG ; rows 96..128 halo
    m0 = np.full((128, 64), NEG, np.float32)
    i = np.arange(32)[:, None]
    qt = np.arange(32)[None, :]
    tri = np.where(qt <= i, 0.0, NEG).astype(np.float32)
    m0[96:128, 0:32] = tri
    m0[96:128, 32:64] = tri

    qf = np.ascontiguousarray(query.transpose(2, 1, 0).astype(np.float32))  # [E, N, L]
    kf = np.ascontiguousarray(key.transpose(2, 1, 0).astype(np.float32))
    vf = np.ascontiguousarray(value.transpose(2, 1, 0).astype(np.float32))

    in_maps = []
    for cidx in range(NCORES):
        l0 = cidx * T
        xq = qf[:, :, l0 : l0 + T].reshape(2, 128, N * T)
        xk = np.zeros((2, 128, N, TL), np.float32)
        xv = np.zeros((2, 128, N, TL), np.float32)
        kfc = kf.reshape(2, 128, N, L)
        vfc = vf.reshape(2, 128, N, L)
        xk[:, :, :, 128:] = kfc[:, :, :, l0 : l0 + T]
        xv[:, :, :, 128:] = vfc[:, :, :, l0 : l0 + T]
        if cidx > 0:
            xk[:, :, :, 96:128] = kfc[:, :, :, l0 - 32 : l0]
            xv[:, :, :, 96:128] = vfc[:, :, :, l0 - 32 : l0]
        wpack = wpack_base.copy()
        if cidx == 0:
            wpack[:, _M0 : _M0 + 64] = NEG
        else:
            wpack[:, _M0 : _M0 + 64] = m0
        in_maps.append(
            {
                "xq": np.ascontiguousarray(xq),
                "xk": np.ascontiguousarray(xk.reshape(2, 128, N * TL)),
                "xv": np.ascontiguousarray(xv.reshape(2, 128, N * TL)),
                "wpack": wpack,
            }
        )
    add_vec = (out_proj_b + bv @ wo.T).astype(np.float32)
    return in_maps, add_vec


def _get_state():
    if "nc" not in _STATE:
        _STATE["nc"] = _build_program()
    return _STATE["nc"]


def kernel(query, key, value, in_proj_w, in_proj_b, out_proj_w, out_proj_b,
           collect_intermediates=0, _trace=False):
    from concourse.bass_utils import run_bass_kernel_spmd

    nc = _get_state()
    in_maps, add_vec = _host_prep(
        np.asarray(query), np.asarray(key), np.asarray(value),
        np.asarray(in_proj_w), np.asarray(in_proj_b),
        np.asarray(out_proj_w), np.asarray(out_proj_b),
    )
    res = run_bass_kernel_spmd(nc, in_maps, list(range(NCORES)), trace=_trace)
    out = np.empty((L, N, E), np.float32)
    for cidx in range(NCORES):
        dev = res.results[cidx]["out"]  # [2, 8, 128, 256]
        shard = dev.transpose(1, 2, 0, 3).reshape(T, N, E)
        out[cidx * T : (cidx + 1) * T] = shard
    out += add_vec
    if _trace:
        _STATE["last_exec_ns"] = res.exec_time_ns
        _STATE["last_res"] = res
    return out



# revision 3
# speedup vs baseline: 1.8128x; 1.8128x over previous
"""ConvolvedAttention (sliding-window causal attention, W=33) on 8 TRN2 NeuronCores.

Sharding: sequence L=8192 split 8 ways (1024 tokens/core), data-parallel over
cores. Host passes each core its query shard plus key/value shards with a
32-token halo on the left; projections are replicated. Each core runs a fused
Bass/Tile kernel in fp16 (fp32 PSUM accumulate): qkv projections -> banded
scores (k-major, query-aligned 128-key supers) -> exp + multiplicative 0/1
band mask -> softmax-sum + AV -> out-projection. Host folds in the output
biases and reassembles.
"""

import numpy as np

# ---- problem constants (hardcoded per contract) ----
L, N, E = 8192, 2, 256
H, HD = 8, 32
WHALF = 32            # window//2 ; attended span = 33 (past only)
NCORES = 8
T = L // NCORES       # 1024 tokens per core
TL = 128 + T          # local tokens per batch entry: 96 pad + 32 halo + 1024
NCORES = 8
NSUP = 9              # supers 0..8 ; super 0 = pad+halo block

# wpack column layout (fp16 cols per partition)
_WQ = 0               # 4 tiles [128,128]  (ki*2+ko)
_WK = 512
_WV = 1024            # 2 tiles [128,256]  (ki)
_WO = 1536            # 2 tiles [128,256]  (g = E_in chunk)
_ONES32 = 2048        # [128,32] all-ones (S-sum lhsT)
_MM01 = 2080          # [128, 2x160] band mask, 2 heads tiled (0/1 fp16)
_M001 = 2400          # [128, 2x32] super-0 mask (pad+halo), 2 heads tiled
_WPCOLS = 2464

_STATE = {}


def _sup_w(s):
    return 32 if s == 0 else (128 if s == NSUP - 1 else 160)


def _build_program():
    import concourse.bacc as bacc
    import concourse.tile as tile
    import concourse.mybir as mybir
    from contextlib import ExitStack

    f32 = mybir.dt.float32
    f16 = mybir.dt.float16
    AF = mybir.ActivationFunctionType

    nc = bacc.Bacc("TRN2", target_bir_lowering=False, debug=False)
    xq_d = nc.declare_dram_parameter("xq", [2, 128, 2 * T], f16, isOutput=False)
    xk_d = nc.declare_dram_parameter("xk", [2, 128, 2 * TL], f16, isOutput=False)
    xv_d = nc.declare_dram_parameter("xv", [2, 128, 2 * TL], f16, isOutput=False)
    wp_d = nc.declare_dram_parameter("wpack", [128, _WPCOLS], f16, isOutput=False)
    wb_d = nc.declare_dram_parameter("wbias", [128, 4], f32, isOutput=False)
    out_d = nc.declare_dram_parameter("out", [2, 8, 128, 256], f16, isOutput=True)

    with ExitStack() as stk:
        tc = stk.enter_context(tile.TileContext(nc))
        sb = stk.enter_context(tc.tile_pool(name="sb", bufs=1))
        sb_probs = stk.enter_context(tc.tile_pool(name="probs", bufs=2))
        sb_praw = stk.enter_context(tc.tile_pool(name="praw", bufs=3))
        sb_tr = stk.enter_context(tc.tile_pool(name="tr", bufs=3))

        # ---- load inputs ----
        wp = sb.tile([128, _WPCOLS], f16, tag="wp")
        nc.sync.dma_start(wp[:], wp_d[:])
        wb = sb.tile([128, 4], f32, tag="wb")
        nc.sync.dma_start(wb[:], wb_d[:])
        xq = []
        xk = []
        xv = []
        for ki in range(2):
            t_q = sb.tile([128, 2 * T], f16, tag=f"xq{ki}", name=f"xq{ki}")
            nc.sync.dma_start(t_q[:], xq_d[ki])
            xq.append(t_q)
            t_k = sb.tile([128, 2 * TL], f16, tag=f"xk{ki}", name=f"xk{ki}")
            nc.sync.dma_start(t_k[:], xk_d[ki])
            xk.append(t_k)
            t_v = sb.tile([128, 2 * TL], f16, tag=f"xv{ki}", name=f"xv{ki}")
            nc.sync.dma_start(t_v[:], xv_d[ki])
            xv.append(t_v)

        q_sb = [sb.tile([128, 2 * T], f16, tag=f"q{ko}", name=f"q{ko}") for ko in range(2)]
        k_sb = [sb.tile([128, 2 * TL], f16, tag=f"k{ko}", name=f"k{ko}") for ko in range(2)]
        v_sb = [sb.tile([128, 256], f16, tag=f"v{b}", name=f"v{b}") for b in range(2 * NSUP)]

        # ---- phase 1: projections ----
        with tc.tile_pool(name="pp", bufs=3, space="PSUM") as pp:
            # q / k projections: out [E_out chunk, tokens]
            for ko in range(2):
                bq_ap = wb[:, ko : ko + 1]
                bk_ap = wb[:, 2 + ko : 3 + ko]
                for g0 in range(0, 2 * T, 512):
                    ps = pp.tile([128, 512], f32, tag="pq", name="pq")
                    for ki in range(2):
                        nc.tensor.matmul(
                            ps[:],
                            wp[:, _WQ + (ki * 2 + ko) * 128 : _WQ + (ki * 2 + ko + 1) * 128],
                            xq[ki][:, g0 : g0 + 512],
                            start=(ki == 0),
                            stop=(ki == 1),
                        )
                    nc.scalar.activation(
                        q_sb[ko][:, g0 : g0 + 512], ps[:], AF.Identity, bias=bq_ap
                    )
                for g0 in range(0, 2 * TL, 512):
                    w = min(512, 2 * TL - g0)
                    ps = pp.tile([128, 512], f32, tag="pq", name="pq")
                    for ki in range(2):
                        nc.tensor.matmul(
                            ps[:, :w],
                            wp[:, _WK + (ki * 2 + ko) * 128 : _WK + (ki * 2 + ko + 1) * 128],
                            xk[ki][:, g0 : g0 + w],
                            start=(ki == 0),
                            stop=(ki == 1),
                        )
                    nc.scalar.activation(
                        k_sb[ko][:, g0 : g0 + w], ps[:, :w], AF.Identity, bias=bk_ap
                    )
            # v projection: out [tokens, E_out]
            for b in range(2 * NSUP):
                ps = pp.tile([128, 256], f32, tag="pv", name="pv")
                for ki in range(2):
                    nc.tensor.matmul(
                        ps[:],
                        xv[ki][:, b * 128 : (b + 1) * 128],
                        wp[:, _WV + ki * 256 : _WV + (ki + 1) * 256],
                        start=(ki == 0),
                        stop=(ki == 1),
                    )
                nc.vector.tensor_copy(v_sb[b][:], ps[:])

        # ---- phase 2: attention ----
        mm01 = wp[:, _MM01 : _MM01 + 320].rearrange("p (t w) -> p t w", t=2)
        m001 = wp[:, _M001 : _M001 + 64].rearrange("p (t w) -> p t w", t=2)
        ones32 = wp[:, _ONES32 : _ONES32 + 32]

        with (
            tc.tile_pool(name="psc", bufs=4, space="PSUM") as psc,
            tc.tile_pool(name="pav", bufs=2, space="PSUM") as pav,
            tc.tile_pool(name="pms", bufs=2, space="PSUM") as pms,
        ):
            for n in range(2):
                probs = {}
                for s in range(NSUP):
                    w = _sup_w(s)
                    qs = 0 if s == 0 else 128 * (s - 1)
                    pr = sb_probs.tile([128, 8 * 160], f16, tag="probs", name="probs")
                    probs[s] = pr
                    pr_r = pr[:, : 8 * w].rearrange("p (a b) -> p a b", a=8)
                    mask = m001 if s == 0 else mm01[:, :, :w]
                    for j in range(4):
                        # bank j: heads j and j+4, both at row group 32j
                        sc = psc.tile([128, 2 * 160], f32, tag="sc", name="sc")
                        for hb in range(2):
                            h = j + 4 * hb
                            ch, hr = hb, 32 * j
                            nc.tensor.matmul(
                                sc[:, hb * w : (hb + 1) * w],
                                k_sb[ch][hr : hr + 32, n * TL + 128 * s : n * TL + 128 * s + 128],
                                q_sb[ch][hr : hr + 32, n * T + qs : n * T + qs + w],
                                start=(hb == 0), stop=(hb == 1),
                                tile_position=(hr, 0), skip_group_check=True,
                            )
                        praw = sb_praw.tile([128, 2 * 160], f16, tag="praw", name="praw")
                        nc.scalar.activation(
                            praw[:, : 2 * w], sc[:, : 2 * w], AF.Exp,
                        )
                        nc.vector.tensor_mul(
                            pr_r[:, j::4, :],
                            praw[:, : 2 * w].rearrange("p (t w) -> p t w", t=2),
                            mask,
                        )
                    if s == 0:
                        continue
                    # finalize query block a = s-1 (queries 128a .. 128a+128)
                    a = s - 1
                    wp_prev = _sup_w(s - 1)
                    wc = min(w, 128)
                    pcur, pprev = probs[s], probs[s - 1]
                    sps = pms.tile([128, 256], f32, tag="ms", name="ms")
                    for h in range(8):
                        hp, hc = 32 * (h % 4), 128 * (h // 4)
                        nc.tensor.matmul(
                            sps[hp : hp + 32, hc : hc + wc], ones32,
                            pcur[:, h * w : h * w + wc],
                            start=True, stop=False, skip_group_check=True,
                            tile_position=(0, hp),
                        )
                        nc.tensor.matmul(
                            sps[hp : hp + 32, hc : hc + 32], ones32,
                            pprev[:, h * wp_prev + wp_prev - 32 : h * wp_prev + wp_prev],
                            start=False, stop=True, skip_group_check=True,
                            tile_position=(0, hp),
                        )
                    s_r = sb_tr.tile([128, 256], f32, tag="sr", name="sr")
                    nc.vector.reciprocal(out=s_r[:], in_=sps[:])
                    avn = []
                    for g in range(2):
                        av = pav.tile([128, 128], f32, tag="av", name="av")
                        for hb in range(4):
                            h = 4 * g + hb
                            hr = 32 * hb
                            nc.tensor.matmul(
                                av[hr : hr + 32, :wc],
                                v_sb[NSUP * n + s][:, 32 * h : 32 * h + 32],
                                pcur[:, h * w : h * w + wc],
                                start=True, stop=False,
                                tile_position=(0, hr), skip_group_check=True,
                            )
                            nc.tensor.matmul(
                                av[hr : hr + 32, :32],
                                v_sb[NSUP * n + s - 1][:, 32 * h : 32 * h + 32],
                                pprev[:, h * wp_prev + wp_prev - 32 : h * wp_prev + wp_prev],
                                start=False, stop=True,
                                tile_position=(0, hr), skip_group_check=True,
                            )
                        t_avn = sb_tr.tile([128, 128], f16, tag="avn", name="avn")
                        nc.vector.tensor_mul(t_avn[:], av[:], s_r[:, 128 * g : 128 * (g + 1)])
                        avn.append(t_avn)
                    op = pms.tile([128, 256], f32, tag="ms", name="ms")
                    for g in range(2):
                        nc.tensor.matmul(
                            op[:], avn[g][:],
                            wp[:, _WO + g * 256 : _WO + (g + 1) * 256],
                            start=(g == 0), stop=(g == 1),
                        )
                    o_sb = sb_tr.tile([128, 256], f16, tag="osb", name="osb")
                    nc.vector.tensor_copy(o_sb[:], op[:])
                    nc.sync.dma_start(out_d[n, a], o_sb[:])
                    del probs[s - 1]
    nc.compile()
    return nc


def _host_prep(query, key, value, in_proj_w, in_proj_b, out_proj_w, out_proj_b):
    """Build per-core input maps + the host-side output bias vector."""
    s = 1.0 / np.sqrt(HD)
    wq = (in_proj_w[:E] * s).astype(np.float32)
    wk = in_proj_w[E : 2 * E].astype(np.float32)
    wv = in_proj_w[2 * E :].astype(np.float32)
    bq = (in_proj_b[:E] * s).astype(np.float32)
    bk = in_proj_b[E : 2 * E].astype(np.float32)
    bv = in_proj_b[2 * E :].astype(np.float32)
    wo = out_proj_w.astype(np.float32)

    wpack_base = np.zeros((128, _WPCOLS), np.float16)
    wqT, wkT = wq.T.copy(), wk.T.copy()   # [E_in, E_out]
    for ki in range(2):
        for ko in range(2):
            wpack_base[:, _WQ + (ki * 2 + ko) * 128 : _WQ + (ki * 2 + ko + 1) * 128] = \
                wqT[ki * 128 : (ki + 1) * 128, ko * 128 : (ko + 1) * 128]
            wpack_base[:, _WK + (ki * 2 + ko) * 128 : _WK + (ki * 2 + ko + 1) * 128] = \
                wkT[ki * 128 : (ki + 1) * 128, ko * 128 : (ko + 1) * 128]
        wpack_base[:, _WV + ki * 256 : _WV + (ki + 1) * 256] = \
            wv.T[ki * 128 : (ki + 1) * 128, :]
        wpack_base[:, _WO + ki * 256 : _WO + (ki + 1) * 256] = \
            wo.T[ki * 128 : (ki + 1) * 128, :]
    wpack_base[:, _ONES32 : _ONES32 + 32] = 1.0
    # band mask [128, 2x160]: valid iff 0 <= c - rho <= WHALF  (0/1)
    rho = np.arange(128)[:, None]
    c = np.arange(160)[None, :]
    band = ((c - rho >= 0) & (c - rho <= WHALF)).astype(np.float16)
    wpack_base[:, _MM01 : _MM01 + 160] = band
    wpack_base[:, _MM01 + 160 : _MM01 + 320] = band

    wbias = np.zeros((128, 4), np.float32)
    for ko in range(2):
        wbias[:, ko] = bq[ko * 128 : (ko + 1) * 128]
        wbias[:, 2 + ko] = bk[ko * 128 : (ko + 1) * 128]

    # super-0 mask [128, 2x32]: rows 0..96 pad -> 0 ; rows 96..128 halo tri
    m0 = np.zeros((128, 64), np.float16)
    i = np.arange(32)[:, None]
    qt = np.arange(32)[None, :]
    tri = (qt <= i).astype(np.float16)
    m0[96:128, 0:32] = tri
    m0[96:128, 32:64] = tri

    qf = np.ascontiguousarray(query.transpose(2, 1, 0).astype(np.float16))  # [E, N, L]
    kf = np.ascontiguousarray(key.transpose(2, 1, 0).astype(np.float16))
    vf = np.ascontiguousarray(value.transpose(2, 1, 0).astype(np.float16))

    in_maps = []
    for cidx in range(NCORES):
        l0 = cidx * T
        xq = qf[:, :, l0 : l0 + T].reshape(2, 128, N * T)
        xk = np.zeros((2, 128, N, TL), np.float16)
        xv = np.zeros((2, 128, N, TL), np.float16)
        kfc = kf.reshape(2, 128, N, L)
        vfc = vf.reshape(2, 128, N, L)
        xk[:, :, :, 128:] = kfc[:, :, :, l0 : l0 + T]
        xv[:, :, :, 128:] = vfc[:, :, :, l0 : l0 + T]
        if cidx > 0:
            xk[:, :, :, 96:128] = kfc[:, :, :, l0 - 32 : l0]
            xv[:, :, :, 96:128] = vfc[:, :, :, l0 - 32 : l0]
        wpack = wpack_base.copy()
        if cidx == 0:
            wpack[:, _M001 : _M001 + 64] = 0.0
        else:
            wpack[:, _M001 : _M001 + 64] = m0
        in_maps.append(
            {
                "xq": np.ascontiguousarray(xq),
                "xk": np.ascontiguousarray(xk.reshape(2, 128, N * TL)),
                "xv": np.ascontiguousarray(xv.reshape(2, 128, N * TL)),
                "wpack": wpack,
                "wbias": wbias,
            }
        )
    add_vec = (out_proj_b + bv @ wo.T).astype(np.float32)
    return in_maps, add_vec


def _get_state():
    if "nc" not in _STATE:
        _STATE["nc"] = _build_program()
    return _STATE["nc"]


def kernel(query, key, value, in_proj_w, in_proj_b, out_proj_w, out_proj_b,
           collect_intermediates=0, _trace=False):
    from concourse.bass_utils import run_bass_kernel_spmd

    nc = _get_state()
    in_maps, add_vec = _host_prep(
        np.asarray(query), np.asarray(key), np.asarray(value),
        np.asarray(in_proj_w), np.asarray(in_proj_b),
        np.asarray(out_proj_w), np.asarray(out_proj_b),
    )
    res = run_bass_kernel_spmd(nc, in_maps, list(range(NCORES)), trace=_trace)
    out = np.empty((L, N, E), np.float32)
    for cidx in range(NCORES):
        dev = res.results[cidx]["out"]  # [2, 8, 128, 256] fp16
        shard = dev.astype(np.float32).transpose(1, 2, 0, 3).reshape(T, N, E)
        out[cidx * T : (cidx + 1) * T] = shard
    out += add_vec
    if _trace:
        _STATE["last_exec_ns"] = res.exec_time_ns
        _STATE["last_res"] = res
    return out


# revision 6
# speedup vs baseline: 2.5711x; 1.4183x over previous
"""ConvolvedAttention (sliding-window causal attention, W=33) on 8 TRN2 NeuronCores.

Sharding: sequence L=8192 split 8 ways (1024 tokens/core), data-parallel over
cores. Host passes each core its query shard plus key/value shards with a
32-token halo on the left; projections are replicated. Each core runs a fused
Bass/Tile kernel in fp16 (fp32 PSUM accumulate): qkv projections -> banded
scores (k-major, query-aligned 128-key supers) -> exp + multiplicative 0/1
band mask -> softmax-sum + AV -> out-projection. Host folds in the output
biases and reassembles.
"""

import numpy as np

# ---- problem constants (hardcoded per contract) ----
L, N, E = 8192, 2, 256
H, HD = 8, 32
WHALF = 32            # window//2 ; attended span = 33 (past only)
NCORES = 8
T = L // NCORES       # 1024 tokens per core
TL = 128 + T          # local tokens per batch entry: 96 pad + 32 halo + 1024
NCORES = 8
NSUP = 9              # supers 0..8 ; super 0 = pad+halo block

# wpack column layout (fp16 cols per partition)
_WQ = 0               # 4 tiles [128,128]  (ki*2+ko)
_WK = 512
_WV = 1024            # 2 tiles [128,256]  (ki)
_WO = 1536            # 2 tiles [128,256]  (g = E_in chunk)
_ONES32 = 2048        # [128,32] all-ones (S-sum lhsT)
_MM01 = 2080          # [128, 2x160] band mask, 2 heads tiled (0/1 fp16)
_M001 = 2400          # [128, 2x32] super-0 mask (pad+halo), 2 heads tiled
_WPCOLS = 2464

_STATE = {}


def _sup_w(s):
    return 32 if s == 0 else (128 if s == NSUP - 1 else 160)


def _build_program():
    import concourse.bacc as bacc
    import concourse.tile as tile
    import concourse.mybir as mybir
    from contextlib import ExitStack

    f32 = mybir.dt.float32
    f16 = mybir.dt.float16
    AF = mybir.ActivationFunctionType

    nc = bacc.Bacc("TRN2", target_bir_lowering=False, debug=False)
    xq_d = nc.declare_dram_parameter("xq", [2, 128, 2 * T], f16, isOutput=False)
    xk_d = nc.declare_dram_parameter("xk", [2, 128, 2 * TL], f16, isOutput=False)
    xv_d = nc.declare_dram_parameter("xv", [2, 128, 2 * TL], f16, isOutput=False)
    wp_d = nc.declare_dram_parameter("wpack", [128, _WPCOLS], f16, isOutput=False)
    wb_d = nc.declare_dram_parameter("wbias", [128, 4], f32, isOutput=False)
    out_d = nc.declare_dram_parameter("out", [2, 8, 128, 256], f16, isOutput=True)

    with ExitStack() as stk:
        tc = stk.enter_context(tile.TileContext(nc))
        sb = stk.enter_context(tc.tile_pool(name="sb", bufs=1))
        sb_probs = stk.enter_context(tc.tile_pool(name="probs", bufs=3))
        sb_praw = stk.enter_context(tc.tile_pool(name="praw", bufs=3))
        sb_tr = stk.enter_context(tc.tile_pool(name="tr", bufs=3))

        # ---- load inputs ----
        wp = sb.tile([128, _WPCOLS], f16, tag="wp")
        nc.sync.dma_start(wp[:], wp_d[:])
        wb = sb.tile([128, 4], f32, tag="wb")
        nc.sync.dma_start(wb[:], wb_d[:])
        xq = []
        xk = []
        xv = []
        for ki in range(2):
            t_q = sb.tile([128, 2 * T], f16, tag=f"xq{ki}", name=f"xq{ki}")
            nc.sync.dma_start(t_q[:], xq_d[ki])
            xq.append(t_q)
            t_k = sb.tile([128, 2 * TL], f16, tag=f"xk{ki}", name=f"xk{ki}")
            nc.sync.dma_start(t_k[:], xk_d[ki])
            xk.append(t_k)
            t_v = sb.tile([128, 2 * TL], f16, tag=f"xv{ki}", name=f"xv{ki}")
            nc.sync.dma_start(t_v[:], xv_d[ki])
            xv.append(t_v)

        q_sb = [sb.tile([128, 2 * T], f16, tag=f"q{ko}", name=f"q{ko}") for ko in range(2)]
        k_sb = [sb.tile([128, 2 * TL], f16, tag=f"k{ko}", name=f"k{ko}") for ko in range(2)]
        v_sb = [sb.tile([128, 256], f16, tag=f"v{b}", name=f"v{b}") for b in range(2 * NSUP)]

        # ---- phase 1: projections ----
        with tc.tile_pool(name="pp", bufs=3, space="PSUM") as pp:
            # q / k projections: out [E_out chunk, tokens]
            for ko in range(2):
                bq_ap = wb[:, ko : ko + 1]
                bk_ap = wb[:, 2 + ko : 3 + ko]
                for g0 in range(0, 2 * T, 512):
                    ps = pp.tile([128, 512], f32, tag="pq", name="pq")
                    for ki in range(2):
                        nc.tensor.matmul(
                            ps[:],
                            wp[:, _WQ + (ki * 2 + ko) * 128 : _WQ + (ki * 2 + ko + 1) * 128],
                            xq[ki][:, g0 : g0 + 512],
                            start=(ki == 0),
                            stop=(ki == 1),
                        )
                    nc.scalar.activation(
                        q_sb[ko][:, g0 : g0 + 512], ps[:], AF.Identity, bias=bq_ap
                    )
                for g0 in range(0, 2 * TL, 512):
                    w = min(512, 2 * TL - g0)
                    ps = pp.tile([128, 512], f32, tag="pq", name="pq")
                    for ki in range(2):
                        nc.tensor.matmul(
                            ps[:, :w],
                            wp[:, _WK + (ki * 2 + ko) * 128 : _WK + (ki * 2 + ko + 1) * 128],
                            xk[ki][:, g0 : g0 + w],
                            start=(ki == 0),
                            stop=(ki == 1),
                        )
                    nc.scalar.activation(
                        k_sb[ko][:, g0 : g0 + w], ps[:, :w], AF.Identity, bias=bk_ap
                    )
            # v projection: out [tokens, E_out]
            for b in range(2 * NSUP):
                ps = pp.tile([128, 256], f32, tag="pv", name="pv")
                for ki in range(2):
                    nc.tensor.matmul(
                        ps[:],
                        xv[ki][:, b * 128 : (b + 1) * 128],
                        wp[:, _WV + ki * 256 : _WV + (ki + 1) * 256],
                        start=(ki == 0),
                        stop=(ki == 1),
                    )
                nc.scalar.copy(v_sb[b][:], ps[:])

        # ---- phase 2: attention ----
        mm01 = wp[:, _MM01 : _MM01 + 320].rearrange("p (t w) -> p t w", t=2)
        m001 = wp[:, _M001 : _M001 + 64].rearrange("p (t w) -> p t w", t=2)
        ones32 = wp[:, _ONES32 : _ONES32 + 32]

        with (
            tc.tile_pool(name="psc", bufs=4, space="PSUM") as psc,
            tc.tile_pool(name="pav", bufs=2, space="PSUM") as pav,
            tc.tile_pool(name="pms", bufs=2, space="PSUM") as pms,
        ):
            for n in range(2):
                probs = {}
                for s in range(NSUP):
                    w = _sup_w(s)
                    qs = 0 if s == 0 else 128 * (s - 1)
                    pr = sb_probs.tile([128, 8 * 160], f16, tag="probs", name="probs")
                    probs[s] = pr
                    pr_r = pr[:, : 8 * w].rearrange("p (a b) -> p a b", a=8)
                    mask = m001 if s == 0 else mm01[:, :, :w]
                    for j in range(4):
                        # bank j: heads j and j+4, both at row group 32j
                        sc = psc.tile([128, 2 * 160], f32, tag="sc", name="sc")
                        for hb in range(2):
                            h = j + 4 * hb
                            ch, hr = hb, 32 * j
                            nc.tensor.matmul(
                                sc[:, hb * w : (hb + 1) * w],
                                k_sb[ch][hr : hr + 32, n * TL + 128 * s : n * TL + 128 * s + 128],
                                q_sb[ch][hr : hr + 32, n * T + qs : n * T + qs + w],
                                start=(hb == 0), stop=(hb == 1),
                                tile_position=(hr, 0), skip_group_check=True,
                            )
                        praw = sb_praw.tile([128, 2 * 160], f16, tag="praw", name="praw")
                        nc.scalar.activation(
                            praw[:, : 2 * w], sc[:, : 2 * w], AF.Exp,
                        )
                        nc.vector.tensor_mul(
                            pr_r[:, j::4, :],
                            praw[:, : 2 * w].rearrange("p (t w) -> p t w", t=2),
                            mask,
                        )
                    if s == 0:
                        continue
                    # finalize query block a = s-1 (queries 128a .. 128a+128)
                    a = s - 1
                    wp_prev = _sup_w(s - 1)
                    wc = min(w, 128)
                    pcur, pprev = probs[s], probs[s - 1]
                    sps = pms.tile([128, 256], f32, tag="ms", name="ms")
                    for h in range(8):
                        hp, hc = 32 * (h % 4), 128 * (h // 4)
                        nc.tensor.matmul(
                            sps[hp : hp + 32, hc : hc + wc], ones32,
                            pcur[:, h * w : h * w + wc],
                            start=True, stop=False, skip_group_check=True,
                            tile_position=(0, hp),
                        )
                        nc.tensor.matmul(
                            sps[hp : hp + 32, hc : hc + 32], ones32,
                            pprev[:, h * wp_prev + wp_prev - 32 : h * wp_prev + wp_prev],
                            start=False, stop=True, skip_group_check=True,
                            tile_position=(0, hp),
                        )
                    s_r = sb_tr.tile([128, 256], f32, tag="sr", name="sr")
                    nc.vector.reciprocal_approx_fast(out=s_r[:], in_=sps[:])
                    avn = []
                    for g in range(2):
                        av = pav.tile([128, 128], f32, tag="av", name="av")
                        for hb in range(4):
                            h = 4 * g + hb
                            hr = 32 * hb
                            nc.tensor.matmul(
                                av[hr : hr + 32, :wc],
                                v_sb[NSUP * n + s][:, 32 * h : 32 * h + 32],
                                pcur[:, h * w : h * w + wc],
                                start=True, stop=False,
                                tile_position=(0, hr), skip_group_check=True,
                            )
                            nc.tensor.matmul(
                                av[hr : hr + 32, :32],
                                v_sb[NSUP * n + s - 1][:, 32 * h : 32 * h + 32],
                                pprev[:, h * wp_prev + wp_prev - 32 : h * wp_prev + wp_prev],
                                start=False, stop=True,
                                tile_position=(0, hr), skip_group_check=True,
                            )
                        t_avn = sb_tr.tile([128, 128], f16, tag="avn", name="avn")
                        nc.vector.tensor_mul(t_avn[:], av[:], s_r[:, 128 * g : 128 * (g + 1)])
                        avn.append(t_avn)
                    op = pms.tile([128, 256], f32, tag="ms", name="ms")
                    for g in range(2):
                        nc.tensor.matmul(
                            op[:], avn[g][:],
                            wp[:, _WO + g * 256 : _WO + (g + 1) * 256],
                            start=(g == 0), stop=(g == 1),
                        )
                    o_sb = sb_tr.tile([128, 256], f16, tag="osb", name="osb")
                    nc.vector.tensor_copy(o_sb[:], op[:])
                    nc.sync.dma_start(out_d[n, a], o_sb[:])
                    del probs[s - 1]
    nc.compile()
    return nc


def _host_prep(query, key, value, in_proj_w, in_proj_b, out_proj_w, out_proj_b):
    """Build per-core input maps + the host-side output bias vector."""
    s = 1.0 / np.sqrt(HD)
    wq = (in_proj_w[:E] * s).astype(np.float32)
    wk = in_proj_w[E : 2 * E].astype(np.float32)
    wv = in_proj_w[2 * E :].astype(np.float32)
    bq = (in_proj_b[:E] * s).astype(np.float32)
    bk = in_proj_b[E : 2 * E].astype(np.float32)
    bv = in_proj_b[2 * E :].astype(np.float32)
    wo = out_proj_w.astype(np.float32)

    wpack_base = np.zeros((128, _WPCOLS), np.float16)
    wqT, wkT = wq.T.copy(), wk.T.copy()   # [E_in, E_out]
    for ki in range(2):
        for ko in range(2):
            wpack_base[:, _WQ + (ki * 2 + ko) * 128 : _WQ + (ki * 2 + ko + 1) * 128] = \
                wqT[ki * 128 : (ki + 1) * 128, ko * 128 : (ko + 1) * 128]
            wpack_base[:, _WK + (ki * 2 + ko) * 128 : _WK + (ki * 2 + ko + 1) * 128] = \
                wkT[ki * 128 : (ki + 1) * 128, ko * 128 : (ko + 1) * 128]
        wpack_base[:, _WV + ki * 256 : _WV + (ki + 1) * 256] = \
            wv.T[ki * 128 : (ki + 1) * 128, :]
        wpack_base[:, _WO + ki * 256 : _WO + (ki + 1) * 256] = \
            wo.T[ki * 128 : (ki + 1) * 128, :]
    wpack_base[:, _ONES32 : _ONES32 + 32] = 1.0
    # band mask [128, 2x160]: valid iff 0 <= c - rho <= WHALF  (0/1)
    rho = np.arange(128)[:, None]
    c = np.arange(160)[None, :]
    band = ((c - rho >= 0) & (c - rho <= WHALF)).astype(np.float16)
    wpack_base[:, _MM01 : _MM01 + 160] = band
    wpack_base[:, _MM01 + 160 : _MM01 + 320] = band

    wbias = np.zeros((128, 4), np.float32)
    for ko in range(2):
        wbias[:, ko] = bq[ko * 128 : (ko + 1) * 128]
        wbias[:, 2 + ko] = bk[ko * 128 : (ko + 1) * 128]

    # super-0 mask [128, 2x32]: rows 0..96 pad -> 0 ; rows 96..128 halo tri
    m0 = np.zeros((128, 64), np.float16)
    i = np.arange(32)[:, None]
    qt = np.arange(32)[None, :]
    tri = (qt <= i).astype(np.float16)
    m0[96:128, 0:32] = tri
    m0[96:128, 32:64] = tri

    qf = np.ascontiguousarray(query.transpose(2, 1, 0).astype(np.float16))  # [E, N, L]
    kf = np.ascontiguousarray(key.transpose(2, 1, 0).astype(np.float16))
    vf = np.ascontiguousarray(value.transpose(2, 1, 0).astype(np.float16))

    in_maps = []
    for cidx in range(NCORES):
        l0 = cidx * T
        xq = qf[:, :, l0 : l0 + T].reshape(2, 128, N * T)
        xk = np.zeros((2, 128, N, TL), np.float16)
        xv = np.zeros((2, 128, N, TL), np.float16)
        kfc = kf.reshape(2, 128, N, L)
        vfc = vf.reshape(2, 128, N, L)
        xk[:, :, :, 128:] = kfc[:, :, :, l0 : l0 + T]
        xv[:, :, :, 128:] = vfc[:, :, :, l0 : l0 + T]
        if cidx > 0:
            xk[:, :, :, 96:128] = kfc[:, :, :, l0 - 32 : l0]
            xv[:, :, :, 96:128] = vfc[:, :, :, l0 - 32 : l0]
        wpack = wpack_base.copy()
        if cidx == 0:
            wpack[:, _M001 : _M001 + 64] = 0.0
        else:
            wpack[:, _M001 : _M001 + 64] = m0
        in_maps.append(
            {
                "xq": np.ascontiguousarray(xq),
                "xk": np.ascontiguousarray(xk.reshape(2, 128, N * TL)),
                "xv": np.ascontiguousarray(xv.reshape(2, 128, N * TL)),
                "wpack": wpack,
                "wbias": wbias,
            }
        )
    add_vec = (out_proj_b + bv @ wo.T).astype(np.float32)
    return in_maps, add_vec


def _get_state():
    if "nc" not in _STATE:
        _STATE["nc"] = _build_program()
    return _STATE["nc"]


def kernel(query, key, value, in_proj_w, in_proj_b, out_proj_w, out_proj_b,
           collect_intermediates=0, _trace=False):
    from concourse.bass_utils import run_bass_kernel_spmd

    nc = _get_state()
    in_maps, add_vec = _host_prep(
        np.asarray(query), np.asarray(key), np.asarray(value),
        np.asarray(in_proj_w), np.asarray(in_proj_b),
        np.asarray(out_proj_w), np.asarray(out_proj_b),
    )
    res = run_bass_kernel_spmd(nc, in_maps, list(range(NCORES)), trace=_trace)
    out = np.empty((L, N, E), np.float32)
    for cidx in range(NCORES):
        dev = res.results[cidx]["out"]  # [2, 8, 128, 256] fp16
        shard = dev.astype(np.float32).transpose(1, 2, 0, 3).reshape(T, N, E)
        out[cidx * T : (cidx + 1) * T] = shard
    out += add_vec
    if _trace:
        _STATE["last_exec_ns"] = res.exec_time_ns
        _STATE["last_res"] = res
    return out


# revision 9
# speedup vs baseline: 2.6618x; 1.0353x over previous
"""ConvolvedAttention (sliding-window causal attention, W=33) on 8 TRN2 NeuronCores.

Sharding: sequence L=8192 split 8 ways (1024 tokens/core), data-parallel over
cores. Host passes each core its query shard plus key/value shards with a
32-token halo on the left; projections are replicated. Each core runs a fused
Bass/Tile kernel in fp16 (fp32 PSUM accumulate): qkv projections -> banded
scores (k-major, query-aligned 128-key supers) -> exp + multiplicative 0/1
band mask -> softmax-sum + AV -> out-projection. Host folds in the output
biases and reassembles.
"""

import numpy as np

# ---- problem constants (hardcoded per contract) ----
L, N, E = 8192, 2, 256
H, HD = 8, 32
WHALF = 32            # window//2 ; attended span = 33 (past only)
NCORES = 8
T = L // NCORES       # 1024 tokens per core
TL = 128 + T          # local tokens per batch entry: 96 pad + 32 halo + 1024
NCORES = 8
NSUP = 9              # supers 0..8 ; super 0 = pad+halo block

# wpack column layout (fp16 cols per partition)
_WQ = 0               # 4 tiles [128,128]  (ki*2+ko)
_WK = 512
_WV = 1024            # 2 tiles [128,256]  (ki)
_WO = 1536            # 2 tiles [128,256]  (g = E_in chunk)
_ONES32 = 2048        # [128,32] all-ones (S-sum lhsT)
_MM01 = 2080          # [128, 2x160] band mask, 2 heads tiled (0/1 fp16)
_M001 = 2400          # [128, 2x32] super-0 mask (pad+halo), 2 heads tiled
_WPCOLS = 2464

_STATE = {}


def _sup_w(s):
    return 32 if s == 0 else (128 if s == NSUP - 1 else 160)


def _build_program():
    import concourse.bacc as bacc
    import concourse.tile as tile
    import concourse.mybir as mybir
    from contextlib import ExitStack

    f32 = mybir.dt.float32
    f16 = mybir.dt.float16
    AF = mybir.ActivationFunctionType

    nc = bacc.Bacc("TRN2", target_bir_lowering=False, debug=False)
    xq_d = nc.declare_dram_parameter("xq", [2, 128, 2 * T], f16, isOutput=False)
    xk_d = nc.declare_dram_parameter("xk", [2, 128, 2 * TL], f16, isOutput=False)
    xv_d = nc.declare_dram_parameter("xv", [2, 128, 2 * TL], f16, isOutput=False)
    wp_d = nc.declare_dram_parameter("wpack", [128, _WPCOLS], f16, isOutput=False)
    wb_d = nc.declare_dram_parameter("wbias", [128, 4], f32, isOutput=False)
    out_d = nc.declare_dram_parameter("out", [2, 8, 128, 256], f16, isOutput=True)

    with ExitStack() as stk:
        tc = stk.enter_context(tile.TileContext(nc))
        sb = stk.enter_context(tc.tile_pool(name="sb", bufs=1))
        sb_probs = stk.enter_context(tc.tile_pool(name="probs", bufs=4))
        sb_praw = stk.enter_context(tc.tile_pool(name="praw", bufs=4))
        sb_tr = stk.enter_context(tc.tile_pool(name="tr", bufs=3))

        # ---- load inputs ----
        wp = sb.tile([128, _WPCOLS], f16, tag="wp")
        nc.sync.dma_start(wp[:], wp_d[:])
        wb = sb.tile([128, 4], f32, tag="wb")
        nc.sync.dma_start(wb[:], wb_d[:])
        xq = []
        xk = []
        xv = []
        for ki in range(2):
            t_q = sb.tile([128, 2 * T], f16, tag=f"xq{ki}", name=f"xq{ki}")
            nc.sync.dma_start(t_q[:, :T], xq_d[ki, :, :T])
            nc.sync.dma_start(t_q[:, T:], xq_d[ki, :, T:])
            xq.append(t_q)
            t_k = sb.tile([128, 2 * TL], f16, tag=f"xk{ki}", name=f"xk{ki}")
            nc.sync.dma_start(t_k[:, :TL], xk_d[ki, :, :TL])
            nc.sync.dma_start(t_k[:, TL:], xk_d[ki, :, TL:])
            xk.append(t_k)
            t_v = sb.tile([128, 2 * TL], f16, tag=f"xv{ki}", name=f"xv{ki}")
            nc.sync.dma_start(t_v[:, :TL], xv_d[ki, :, :TL])
            nc.sync.dma_start(t_v[:, TL:], xv_d[ki, :, TL:])
            xv.append(t_v)

        q_sb = [sb.tile([128, 2 * T], f16, tag=f"q{ko}", name=f"q{ko}") for ko in range(2)]
        k_sb = [sb.tile([128, 2 * TL], f16, tag=f"k{ko}", name=f"k{ko}") for ko in range(2)]
        v_sb = [sb.tile([128, 256], f16, tag=f"v{b}", name=f"v{b}") for b in range(2 * NSUP)]

        # ---- phase 1: projections ----
        with tc.tile_pool(name="pp", bufs=3, space="PSUM") as pp:
            # q / k projections: out [E_out chunk, tokens]
            for ko in range(2):
                bq_ap = wb[:, ko : ko + 1]
                bk_ap = wb[:, 2 + ko : 3 + ko]
                for g0 in range(0, 2 * T, 512):
                    ps = pp.tile([128, 512], f32, tag="pq", name="pq")
                    for ki in range(2):
                        nc.tensor.matmul(
                            ps[:],
                            wp[:, _WQ + (ki * 2 + ko) * 128 : _WQ + (ki * 2 + ko + 1) * 128],
                            xq[ki][:, g0 : g0 + 512],
                            start=(ki == 0),
                            stop=(ki == 1),
                        )
                    nc.scalar.activation(
                        q_sb[ko][:, g0 : g0 + 512], ps[:], AF.Identity, bias=bq_ap
                    )
                for g0 in range(0, 2 * TL, 512):
                    w = min(512, 2 * TL - g0)
                    ps = pp.tile([128, 512], f32, tag="pq", name="pq")
                    for ki in range(2):
                        nc.tensor.matmul(
                            ps[:, :w],
                            wp[:, _WK + (ki * 2 + ko) * 128 : _WK + (ki * 2 + ko + 1) * 128],
                            xk[ki][:, g0 : g0 + w],
                            start=(ki == 0),
                            stop=(ki == 1),
                        )
                    nc.scalar.activation(
                        k_sb[ko][:, g0 : g0 + w], ps[:, :w], AF.Identity, bias=bk_ap
                    )
            # v projection: out [tokens, E_out]
            for b in range(2 * NSUP):
                ps = pp.tile([128, 256], f32, tag="pv", name="pv")
                for ki in range(2):
                    nc.tensor.matmul(
                        ps[:],
                        xv[ki][:, b * 128 : (b + 1) * 128],
                        wp[:, _WV + ki * 256 : _WV + (ki + 1) * 256],
                        start=(ki == 0),
                        stop=(ki == 1),
                    )
                nc.vector.tensor_copy(v_sb[b][:], ps[:])

        # ---- phase 2: attention ----
        mm01 = wp[:, _MM01 : _MM01 + 320].rearrange("p (t w) -> p t w", t=2)
        m001 = wp[:, _M001 : _M001 + 64].rearrange("p (t w) -> p t w", t=2)
        ones32 = wp[:, _ONES32 : _ONES32 + 32]

        with (
            tc.tile_pool(name="psc", bufs=4, space="PSUM") as psc,
            tc.tile_pool(name="pav", bufs=2, space="PSUM") as pav,
            tc.tile_pool(name="pms", bufs=2, space="PSUM") as pms,
        ):
            for n in range(2):
                probs = {}
                for s in range(NSUP):
                    w = _sup_w(s)
                    qs = 0 if s == 0 else 128 * (s - 1)
                    pr = sb_probs.tile([128, 8 * 160], f16, tag="probs", name="probs")
                    probs[s] = pr
                    pr_r = pr[:, : 8 * w].rearrange("p (a b) -> p a b", a=8)
                    mask = m001 if s == 0 else mm01[:, :, :w]
                    for j in range(4):
                        # bank j: heads j and j+4, both at row group 32j
                        sc = psc.tile([128, 2 * 160], f32, tag="sc", name="sc")
                        for hb in range(2):
                            h = j + 4 * hb
                            ch, hr = hb, 32 * j
                            nc.tensor.matmul(
                                sc[:, hb * w : (hb + 1) * w],
                                k_sb[ch][hr : hr + 32, n * TL + 128 * s : n * TL + 128 * s + 128],
                                q_sb[ch][hr : hr + 32, n * T + qs : n * T + qs + w],
                                start=(hb == 0), stop=(hb == 1),
                                tile_position=(hr, 0), skip_group_check=True,
                            )
                        praw = sb_praw.tile([128, 2 * 160], f16, tag="praw", name="praw")
                        nc.scalar.activation(
                            praw[:, : 2 * w], sc[:, : 2 * w], AF.Exp,
                        )
                        nc.vector.tensor_mul(
                            pr_r[:, j::4, :],
                            praw[:, : 2 * w].rearrange("p (t w) -> p t w", t=2),
                            mask,
                        )
                    if s == 0:
                        continue
                    # finalize query block a = s-1 (queries 128a .. 128a+128)
                    a = s - 1
                    wp_prev = _sup_w(s - 1)
                    wc = min(w, 128)
                    pcur, pprev = probs[s], probs[s - 1]
                    sps = pms.tile([128, 256], f32, tag="ms", name="ms")
                    for h in range(8):
                        hp, hc = 32 * (h % 4), 128 * (h // 4)
                        nc.tensor.matmul(
                            sps[hp : hp + 32, hc : hc + wc], ones32,
                            pcur[:, h * w : h * w + wc],
                            start=True, stop=False, skip_group_check=True,
                            tile_position=(0, hp),
                        )
                        nc.tensor.matmul(
                            sps[hp : hp + 32, hc : hc + 32], ones32,
                            pprev[:, h * wp_prev + wp_prev - 32 : h * wp_prev + wp_prev],
                            start=False, stop=True, skip_group_check=True,
                            tile_position=(0, hp),
                        )
                    s_r = sb_tr.tile([128, 256], f32, tag="sr", name="sr")
                    nc.vector.reciprocal_approx_fast(out=s_r[:], in_=sps[:])
                    avn = []
                    for g in range(2):
                        av = pav.tile([128, 128], f32, tag="av", name="av")
                        for hb in range(4):
                            h = 4 * g + hb
                            hr = 32 * hb
                            nc.tensor.matmul(
                                av[hr : hr + 32, :wc],
                                v_sb[NSUP * n + s][:, 32 * h : 32 * h + 32],
                                pcur[:, h * w : h * w + wc],
                                start=True, stop=False,
                                tile_position=(0, hr), skip_group_check=True,
                            )
                            nc.tensor.matmul(
                                av[hr : hr + 32, :32],
                                v_sb[NSUP * n + s - 1][:, 32 * h : 32 * h + 32],
                                pprev[:, h * wp_prev + wp_prev - 32 : h * wp_prev + wp_prev],
                                start=False, stop=True,
                                tile_position=(0, hr), skip_group_check=True,
                            )
                        t_avn = sb_tr.tile([128, 128], f16, tag="avn", name="avn")
                        nc.vector.tensor_mul(t_avn[:], av[:], s_r[:, 128 * g : 128 * (g + 1)])
                        avn.append(t_avn)
                    op = pms.tile([128, 256], f32, tag="ms", name="ms")
                    for g in range(2):
                        nc.tensor.matmul(
                            op[:], avn[g][:],
                            wp[:, _WO + g * 256 : _WO + (g + 1) * 256],
                            start=(g == 0), stop=(g == 1),
                        )
                    o_sb = sb_tr.tile([128, 256], f16, tag="osb", name="osb")
                    nc.vector.tensor_copy(o_sb[:], op[:])
                    nc.sync.dma_start(out_d[n, a], o_sb[:])
                    del probs[s - 1]
    nc.compile()
    return nc


def _host_prep(query, key, value, in_proj_w, in_proj_b, out_proj_w, out_proj_b):
    """Build per-core input maps + the host-side output bias vector."""
    s = 1.0 / np.sqrt(HD)
    wq = (in_proj_w[:E] * s).astype(np.float32)
    wk = in_proj_w[E : 2 * E].astype(np.float32)
    wv = in_proj_w[2 * E :].astype(np.float32)
    bq = (in_proj_b[:E] * s).astype(np.float32)
    bk = in_proj_b[E : 2 * E].astype(np.float32)
    bv = in_proj_b[2 * E :].astype(np.float32)
    wo = out_proj_w.astype(np.float32)

    wpack_base = np.zeros((128, _WPCOLS), np.float16)
    wqT, wkT = wq.T.copy(), wk.T.copy()   # [E_in, E_out]
    for ki in range(2):
        for ko in range(2):
            wpack_base[:, _WQ + (ki * 2 + ko) * 128 : _WQ + (ki * 2 + ko + 1) * 128] = \
                wqT[ki * 128 : (ki + 1) * 128, ko * 128 : (ko + 1) * 128]
            wpack_base[:, _WK + (ki * 2 + ko) * 128 : _WK + (ki * 2 + ko + 1) * 128] = \
                wkT[ki * 128 : (ki + 1) * 128, ko * 128 : (ko + 1) * 128]
        wpack_base[:, _WV + ki * 256 : _WV + (ki + 1) * 256] = \
            wv.T[ki * 128 : (ki + 1) * 128, :]
        wpack_base[:, _WO + ki * 256 : _WO + (ki + 1) * 256] = \
            wo.T[ki * 128 : (ki + 1) * 128, :]
    wpack_base[:, _ONES32 : _ONES32 + 32] = 1.0
    # band mask [128, 2x160]: valid iff 0 <= c - rho <= WHALF  (0/1)
    rho = np.arange(128)[:, None]
    c = np.arange(160)[None, :]
    band = ((c - rho >= 0) & (c - rho <= WHALF)).astype(np.float16)
    wpack_base[:, _MM01 : _MM01 + 160] = band
    wpack_base[:, _MM01 + 160 : _MM01 + 320] = band

    wbias = np.zeros((128, 4), np.float32)
    for ko in range(2):
        wbias[:, ko] = bq[ko * 128 : (ko + 1) * 128]
        wbias[:, 2 + ko] = bk[ko * 128 : (ko + 1) * 128]

    # super-0 mask [128, 2x32]: rows 0..96 pad -> 0 ; rows 96..128 halo tri
    m0 = np.zeros((128, 64), np.float16)
    i = np.arange(32)[:, None]
    qt = np.arange(32)[None, :]
    tri = (qt <= i).astype(np.float16)
    m0[96:128, 0:32] = tri
    m0[96:128, 32:64] = tri

    qf = np.ascontiguousarray(query.transpose(2, 1, 0).astype(np.float16))  # [E, N, L]
    kf = np.ascontiguousarray(key.transpose(2, 1, 0).astype(np.float16))
    vf = np.ascontiguousarray(value.transpose(2, 1, 0).astype(np.float16))

    in_maps = []
    for cidx in range(NCORES):
        l0 = cidx * T
        xq = qf[:, :, l0 : l0 + T].reshape(2, 128, N * T)
        xk = np.zeros((2, 128, N, TL), np.float16)
        xv = np.zeros((2, 128, N, TL), np.float16)
        kfc = kf.reshape(2, 128, N, L)
        vfc = vf.reshape(2, 128, N, L)
        xk[:, :, :, 128:] = kfc[:, :, :, l0 : l0 + T]
        xv[:, :, :, 128:] = vfc[:, :, :, l0 : l0 + T]
        if cidx > 0:
            xk[:, :, :, 96:128] = kfc[:, :, :, l0 - 32 : l0]
            xv[:, :, :, 96:128] = vfc[:, :, :, l0 - 32 : l0]
        wpack = wpack_base.copy()
        if cidx == 0:
            wpack[:, _M001 : _M001 + 64] = 0.0
        else:
            wpack[:, _M001 : _M001 + 64] = m0
        in_maps.append(
            {
                "xq": np.ascontiguousarray(xq),
                "xk": np.ascontiguousarray(xk.reshape(2, 128, N * TL)),
                "xv": np.ascontiguousarray(xv.reshape(2, 128, N * TL)),
                "wpack": wpack,
                "wbias": wbias,
            }
        )
    add_vec = (out_proj_b + bv @ wo.T).astype(np.float32)
    return in_maps, add_vec


def _get_state():
    if "nc" not in _STATE:
        _STATE["nc"] = _build_program()
    return _STATE["nc"]


def kernel(query, key, value, in_proj_w, in_proj_b, out_proj_w, out_proj_b,
           collect_intermediates=0, _trace=False):
    from concourse.bass_utils import run_bass_kernel_spmd

    nc = _get_state()
    in_maps, add_vec = _host_prep(
        np.asarray(query), np.asarray(key), np.asarray(value),
        np.asarray(in_proj_w), np.asarray(in_proj_b),
        np.asarray(out_proj_w), np.asarray(out_proj_b),
    )
    res = run_bass_kernel_spmd(nc, in_maps, list(range(NCORES)), trace=_trace)
    out = np.empty((L, N, E), np.float32)
    for cidx in range(NCORES):
        dev = res.results[cidx]["out"]  # [2, 8, 128, 256] fp16
        shard = dev.astype(np.float32).transpose(1, 2, 0, 3).reshape(T, N, E)
        out[cidx * T : (cidx + 1) * T] = shard
    out += add_vec
    if _trace:
        _STATE["last_exec_ns"] = res.exec_time_ns
        _STATE["last_res"] = res
    return out


# revision 14
# speedup vs baseline: 2.7601x; 1.0369x over previous
"""ConvolvedAttention (sliding-window causal attention, W=33) on 8 TRN2 NeuronCores.

Sharding: sequence L=8192 split 8 ways (1024 tokens/core), data-parallel over
cores. Host passes each core its query shard plus key/value shards with a
32-token halo on the left; projections are replicated. Each core runs a fused
Bass/Tile kernel in fp16 (fp32 PSUM accumulate): qkv projections -> banded
scores (k-major, query-aligned 128-key supers) -> exp + multiplicative 0/1
band mask -> softmax-sum + AV -> out-projection. Host folds in the output
biases and reassembles.
"""

import numpy as np

# ---- problem constants (hardcoded per contract) ----
L, N, E = 8192, 2, 256
H, HD = 8, 32
WHALF = 32            # window//2 ; attended span = 33 (past only)
NCORES = 8
T = L // NCORES       # 1024 tokens per core
TL = 128 + T          # local tokens per batch entry: 96 pad + 32 halo + 1024
NCORES = 8
NSUP = 9              # supers 0..8 ; super 0 = pad+halo block

# wpack column layout (fp16 cols per partition)
_WQ = 0               # 4 tiles [128,128]  (ki*2+ko)
_WK = 512
_WV = 1024            # 2 tiles [128,256]  (ki)
_WO = 1536            # 2 tiles [128,256]  (g = E_in chunk)
_ONES32 = 2048        # [128,32] all-ones (S-sum lhsT)
_MM01 = 2080          # [128, 2x160] band mask, 2 heads tiled (0/1 fp16)
_M001 = 2400          # [128, 2x32] super-0 mask (pad+halo), 2 heads tiled
_WPCOLS = 2464

_STATE = {}


def _sup_w(s):
    return 32 if s == 0 else (128 if s == NSUP - 1 else 160)


def _build_program():
    import concourse.bacc as bacc
    import concourse.tile as tile
    import concourse.mybir as mybir
    from contextlib import ExitStack

    f32 = mybir.dt.float32
    f16 = mybir.dt.float16
    AF = mybir.ActivationFunctionType

    nc = bacc.Bacc("TRN2", target_bir_lowering=False, debug=False)
    xq_d = nc.declare_dram_parameter("xq", [2, 128, 2 * T], f16, isOutput=False)
    xk_d = nc.declare_dram_parameter("xk", [2, 128, 2 * TL], f16, isOutput=False)
    xv_d = nc.declare_dram_parameter("xv", [2, 128, 2 * TL], f16, isOutput=False)
    wp_d = nc.declare_dram_parameter("wpack", [128, _WPCOLS], f16, isOutput=False)
    wb_d = nc.declare_dram_parameter("wbias", [128, 4], f32, isOutput=False)
    out_d = nc.declare_dram_parameter("out", [2, 8, 128, 256], f16, isOutput=True)

    with ExitStack() as stk:
        tc = stk.enter_context(tile.TileContext(nc))
        sb = stk.enter_context(tc.tile_pool(name="sb", bufs=1))
        sb_probs = stk.enter_context(tc.tile_pool(name="probs", bufs=6))
        sb_praw = stk.enter_context(tc.tile_pool(name="praw", bufs=6))
        sb_tr = stk.enter_context(tc.tile_pool(name="tr", bufs=4))

        # ---- load inputs ----
        wp = sb.tile([128, _WPCOLS], f16, tag="wp")
        nc.sync.dma_start(wp[:], wp_d[:])
        wb = sb.tile([128, 4], f32, tag="wb")
        nc.sync.dma_start(wb[:], wb_d[:])
        xq = []
        xk = []
        xv = []
        for ki in range(2):
            t_q = sb.tile([128, 2 * T], f16, tag=f"xq{ki}", name=f"xq{ki}")
            for c0 in range(0, 2 * T, 512):
                nc.sync.dma_start(t_q[:, c0 : c0 + 512], xq_d[ki, :, c0 : c0 + 512])
            xq.append(t_q)
        for ki in range(2):
            t_k = sb.tile([128, 2 * TL], f16, tag=f"xk{ki}", name=f"xk{ki}")
            for c0 in range(0, 2 * TL, 576):
                nc.sync.dma_start(t_k[:, c0 : c0 + 576], xk_d[ki, :, c0 : c0 + 576])
            xk.append(t_k)
        for ki in range(2):
            t_v = sb.tile([128, 2 * TL], f16, tag=f"xv{ki}", name=f"xv{ki}")
            for c0 in range(0, 2 * TL, 576):
                nc.sync.dma_start(t_v[:, c0 : c0 + 576], xv_d[ki, :, c0 : c0 + 576])
            xv.append(t_v)

        q_sb = [sb.tile([128, 2 * T], f16, tag=f"q{ko}", name=f"q{ko}") for ko in range(2)]
        k_sb = [sb.tile([128, 2 * TL], f16, tag=f"k{ko}", name=f"k{ko}") for ko in range(2)]
        v_sb = [sb.tile([128, 256], f16, tag=f"v{b}", name=f"v{b}") for b in range(2 * NSUP)]

        # ---- phase 1: projections ----
        with tc.tile_pool(name="pp", bufs=3, space="PSUM") as pp:
            # q / k projections: out [E_out chunk, tokens]
            for ko in range(2):
                bq_ap = wb[:, ko : ko + 1]
                bk_ap = wb[:, 2 + ko : 3 + ko]
                for g0 in range(0, 2 * T, 512):
                    ps = pp.tile([128, 512], f32, tag="pq", name="pq")
                    for ki in range(2):
                        nc.tensor.matmul(
                            ps[:],
                            wp[:, _WQ + (ki * 2 + ko) * 128 : _WQ + (ki * 2 + ko + 1) * 128],
                            xq[ki][:, g0 : g0 + 512],
                            start=(ki == 0),
                            stop=(ki == 1),
                        )
                    nc.scalar.activation(
                        q_sb[ko][:, g0 : g0 + 512], ps[:], AF.Identity, bias=bq_ap
                    )
                for g0 in range(0, 2 * TL, 512):
                    w = min(512, 2 * TL - g0)
                    ps = pp.tile([128, 512], f32, tag="pq", name="pq")
                    for ki in range(2):
                        nc.tensor.matmul(
                            ps[:, :w],
                            wp[:, _WK + (ki * 2 + ko) * 128 : _WK + (ki * 2 + ko + 1) * 128],
                            xk[ki][:, g0 : g0 + w],
                            start=(ki == 0),
                            stop=(ki == 1),
                        )
                    nc.scalar.activation(
                        k_sb[ko][:, g0 : g0 + w], ps[:, :w], AF.Identity, bias=bk_ap
                    )
            # v projection: out [tokens, E_out]
            for b in range(2 * NSUP):
                ps = pp.tile([128, 256], f32, tag="pv", name="pv")
                for ki in range(2):
                    nc.tensor.matmul(
                        ps[:],
                        xv[ki][:, b * 128 : (b + 1) * 128],
                        wp[:, _WV + ki * 256 : _WV + (ki + 1) * 256],
                        start=(ki == 0),
                        stop=(ki == 1),
                    )
                nc.vector.tensor_copy(v_sb[b][:], ps[:])

        # ---- phase 2: attention ----
        mm01 = wp[:, _MM01 : _MM01 + 320].rearrange("p (t w) -> p t w", t=2)
        m001 = wp[:, _M001 : _M001 + 64].rearrange("p (t w) -> p t w", t=2)
        ones32 = wp[:, _ONES32 : _ONES32 + 32]

        with (
            tc.tile_pool(name="psc", bufs=4, space="PSUM") as psc,
            tc.tile_pool(name="pav", bufs=2, space="PSUM") as pav,
            tc.tile_pool(name="pms", bufs=2, space="PSUM") as pms,
        ):
            probs = [{}, {}]
            for s in range(NSUP):
                w = _sup_w(s)
                qs = 0 if s == 0 else 128 * (s - 1)
                for n in range(2):
                    pr = sb_probs.tile([128, 8 * 160], f16, tag="probs", name="probs")
                    probs[n][s] = pr
                    pr_r = pr[:, : 8 * w].rearrange("p (a b) -> p a b", a=8)
                    mask = m001 if s == 0 else mm01[:, :, :w]
                    for j in range(4):
                        # bank j: heads j and j+4, both at row group 32j
                        sc = psc.tile([128, 2 * 160], f32, tag="sc", name="sc")
                        for hb in range(2):
                            h = j + 4 * hb
                            ch, hr = hb, 32 * j
                            nc.tensor.matmul(
                                sc[:, hb * w : (hb + 1) * w],
                                k_sb[ch][hr : hr + 32, n * TL + 128 * s : n * TL + 128 * s + 128],
                                q_sb[ch][hr : hr + 32, n * T + qs : n * T + qs + w],
                                start=(hb == 0), stop=(hb == 1),
                                tile_position=(hr, 0), skip_group_check=True,
                            )
                        praw = sb_praw.tile([128, 2 * 160], f16, tag="praw", name="praw")
                        nc.scalar.activation(
                            praw[:, : 2 * w], sc[:, : 2 * w], AF.Exp,
                        )
                        nc.vector.tensor_mul(
                            pr_r[:, j::4, :],
                            praw[:, : 2 * w].rearrange("p (t w) -> p t w", t=2),
                            mask,
                        )
                if s == 0:
                    continue
                # finalize query block a = s-1 (queries 128a .. 128a+128)
                a = s - 1
                wp_prev = _sup_w(s - 1)
                wc = min(w, 128)
                for n in range(2):
                    pcur, pprev = probs[n][s], probs[n][s - 1]
                    sps = pms.tile([128, 256], f32, tag="ms", name="ms")
                    for h in range(8):
                        hp, hc = 32 * (h % 4), 128 * (h // 4)
                        nc.tensor.matmul(
                            sps[hp : hp + 32, hc : hc + wc], ones32,
                            pcur[:, h * w : h * w + wc],
                            start=True, stop=False, skip_group_check=True,
                            tile_position=(0, hp),
                        )
                        nc.tensor.matmul(
                            sps[hp : hp + 32, hc : hc + 32], ones32,
                            pprev[:, h * wp_prev + wp_prev - 32 : h * wp_prev + wp_prev],
                            start=False, stop=True, skip_group_check=True,
                            tile_position=(0, hp),
                        )
                    s_r = sb_tr.tile([128, 256], f32, tag="sr", name="sr")
                    nc.vector.reciprocal_approx_fast(out=s_r[:], in_=sps[:])
                    avn = []
                    for g in range(2):
                        av = pav.tile([128, 128], f32, tag="av", name="av")
                        for hb in range(4):
                            h = 4 * g + hb
                            hr = 32 * hb
                            nc.tensor.matmul(
                                av[hr : hr + 32, :wc],
                                v_sb[NSUP * n + s][:, 32 * h : 32 * h + 32],
                                pcur[:, h * w : h * w + wc],
                                start=True, stop=False,
                                tile_position=(0, hr), skip_group_check=True,
                            )
                            nc.tensor.matmul(
                                av[hr : hr + 32, :32],
                                v_sb[NSUP * n + s - 1][:, 32 * h : 32 * h + 32],
                                pprev[:, h * wp_prev + wp_prev - 32 : h * wp_prev + wp_prev],
                                start=False, stop=True,
                                tile_position=(0, hr), skip_group_check=True,
                            )
                        t_avn = sb_tr.tile([128, 128], f16, tag="avn", name="avn")
                        nc.vector.tensor_mul(t_avn[:], av[:], s_r[:, 128 * g : 128 * (g + 1)])
                        avn.append(t_avn)
                    op = pms.tile([128, 256], f32, tag="ms", name="ms")
                    for g in range(2):
                        nc.tensor.matmul(
                            op[:], avn[g][:],
                            wp[:, _WO + g * 256 : _WO + (g + 1) * 256],
                            start=(g == 0), stop=(g == 1),
                        )
                    o_sb = sb_tr.tile([128, 256], f16, tag="osb", name="osb")
                    nc.vector.tensor_copy(o_sb[:], op[:])
                    nc.sync.dma_start(out_d[n, a], o_sb[:])
                    del probs[n][s - 1]
    nc.compile()
    return nc


def _host_prep(query, key, value, in_proj_w, in_proj_b, out_proj_w, out_proj_b):
    """Build per-core input maps + the host-side output bias vector."""
    s = 1.0 / np.sqrt(HD)
    wq = (in_proj_w[:E] * s).astype(np.float32)
    wk = in_proj_w[E : 2 * E].astype(np.float32)
    wv = in_proj_w[2 * E :].astype(np.float32)
    bq = (in_proj_b[:E] * s).astype(np.float32)
    bk = in_proj_b[E : 2 * E].astype(np.float32)
    bv = in_proj_b[2 * E :].astype(np.float32)
    wo = out_proj_w.astype(np.float32)

    wpack_base = np.zeros((128, _WPCOLS), np.float16)
    wqT, wkT = wq.T.copy(), wk.T.copy()   # [E_in, E_out]
    for ki in range(2):
        for ko in range(2):
            wpack_base[:, _WQ + (ki * 2 + ko) * 128 : _WQ + (ki * 2 + ko + 1) * 128] = \
                wqT[ki * 128 : (ki + 1) * 128, ko * 128 : (ko + 1) * 128]
            wpack_base[:, _WK + (ki * 2 + ko) * 128 : _WK + (ki * 2 + ko + 1) * 128] = \
                wkT[ki * 128 : (ki + 1) * 128, ko * 128 : (ko + 1) * 128]
        wpack_base[:, _WV + ki * 256 : _WV + (ki + 1) * 256] = \
            wv.T[ki * 128 : (ki + 1) * 128, :]
        wpack_base[:, _WO + ki * 256 : _WO + (ki + 1) * 256] = \
            wo.T[ki * 128 : (ki + 1) * 128, :]
    wpack_base[:, _ONES32 : _ONES32 + 32] = 1.0
    # band mask [128, 2x160]: valid iff 0 <= c - rho <= WHALF  (0/1)
    rho = np.arange(128)[:, None]
    c = np.arange(160)[None, :]
    band = ((c - rho >= 0) & (c - rho <= WHALF)).astype(np.float16)
    wpack_base[:, _MM01 : _MM01 + 160] = band
    wpack_base[:, _MM01 + 160 : _MM01 + 320] = band

    wbias = np.zeros((128, 4), np.float32)
    for ko in range(2):
        wbias[:, ko] = bq[ko * 128 : (ko + 1) * 128]
        wbias[:, 2 + ko] = bk[ko * 128 : (ko + 1) * 128]

    # super-0 mask [128, 2x32]: rows 0..96 pad -> 0 ; rows 96..128 halo tri
    m0 = np.zeros((128, 64), np.float16)
    i = np.arange(32)[:, None]
    qt = np.arange(32)[None, :]
    tri = (qt <= i).astype(np.float16)
    m0[96:128, 0:32] = tri
    m0[96:128, 32:64] = tri

    qf = np.ascontiguousarray(query.transpose(2, 1, 0).astype(np.float16))  # [E, N, L]
    kf = np.ascontiguousarray(key.transpose(2, 1, 0).astype(np.float16))
    vf = np.ascontiguousarray(value.transpose(2, 1, 0).astype(np.float16))

    in_maps = []
    for cidx in range(NCORES):
        l0 = cidx * T
        xq = qf[:, :, l0 : l0 + T].reshape(2, 128, N * T)
        xk = np.zeros((2, 128, N, TL), np.float16)
        xv = np.zeros((2, 128, N, TL), np.float16)
        kfc = kf.reshape(2, 128, N, L)
        vfc = vf.reshape(2, 128, N, L)
        xk[:, :, :, 128:] = kfc[:, :, :, l0 : l0 + T]
        xv[:, :, :, 128:] = vfc[:, :, :, l0 : l0 + T]
        if cidx > 0:
            xk[:, :, :, 96:128] = kfc[:, :, :, l0 - 32 : l0]
            xv[:, :, :, 96:128] = vfc[:, :, :, l0 - 32 : l0]
        wpack = wpack_base.copy()
        if cidx == 0:
            wpack[:, _M001 : _M001 + 64] = 0.0
        else:
            wpack[:, _M001 : _M001 + 64] = m0
        in_maps.append(
            {
                "xq": np.ascontiguousarray(xq),
                "xk": np.ascontiguousarray(xk.reshape(2, 128, N * TL)),
                "xv": np.ascontiguousarray(xv.reshape(2, 128, N * TL)),
                "wpack": wpack,
                "wbias": wbias,
            }
        )
    add_vec = (out_proj_b + bv @ wo.T).astype(np.float32)
    return in_maps, add_vec


def _get_state():
    if "nc" not in _STATE:
        _STATE["nc"] = _build_program()
    return _STATE["nc"]


def kernel(query, key, value, in_proj_w, in_proj_b, out_proj_w, out_proj_b,
           collect_intermediates=0, _trace=False):
    from concourse.bass_utils import run_bass_kernel_spmd

    nc = _get_state()
    in_maps, add_vec = _host_prep(
        np.asarray(query), np.asarray(key), np.asarray(value),
        np.asarray(in_proj_w), np.asarray(in_proj_b),
        np.asarray(out_proj_w), np.asarray(out_proj_b),
    )
    res = run_bass_kernel_spmd(nc, in_maps, list(range(NCORES)), trace=_trace)
    out = np.empty((L, N, E), np.float32)
    for cidx in range(NCORES):
        dev = res.results[cidx]["out"]  # [2, 8, 128, 256] fp16
        shard = dev.astype(np.float32).transpose(1, 2, 0, 3).reshape(T, N, E)
        out[cidx * T : (cidx + 1) * T] = shard
    out += add_vec
    if _trace:
        _STATE["last_exec_ns"] = res.exec_time_ns
        _STATE["last_res"] = res
    return out
